# revision 14
# baseline (speedup 1.0000x reference)
"""Trainium2 Bass kernel for nn_ClassConditionalDriftingLoss.

Math per class c (G gen rows, P pos rows, D=64, T=G+P targets):
  d2[t,g]  = ||x_t||^2 + ||y_g||^2 - 2 x_t.y_g          (x=targets, y=gen)
  k        = exp(-2.5*sqrt(d2)),  k[diag]=0             (dist normalized by sqrt(D)=8,
                                                         TEMP=0.05 -> exp(-20*dist/8*... ) = exp(-2.5*sqrt(d2)))
  row[g]   = sum_t k[g,t];  col[t] = sum_g k[g,t]
  nk       = k * min(rsqrt(row[g])*rsqrt(col[t]), 1e6)  (== k / sqrt(max(row*col,1e-12)))
  s_gen[g] = sum_{t<G} nk ; s_pos[g] = sum_{t>=G} nk
  M_pos    = nk[:,G:] @ pos ; M_neg = nk[:,:G] @ gen
  V        = s_gen*M_pos - s_pos*M_neg
  loss    += sum(V^2);  drift += sum_g ||V[g]||

The kernel matrix is held transposed (Kt[t,g], t on partitions) in SBUF bf16.
One class per NeuronCore (8 classes / 8 cores), no collectives; host sums the
8 scalar pairs.

exp/ln only (single ACT table set `natural_log_exp_and_others`):
  sqrt(z) = exp(0.5*ln(z)); rsqrt(z) = exp(-0.5*ln(z)).
Diagonal masking: a BIG*I matmul accumulated onto the d2 PSUM drives those
entries to exp(-sqrt(6.25e9)) == 0.
"""

import sys

for _p in ("/opt/trn_rl_repo", "/root/.axon_site/_ro/trn_rl_repo"):
    if _p not in sys.path:
        sys.path.insert(0, _p)

import math

import ml_dtypes
import numpy as np

C = 8
BIG = 1.0e9  # added to diagonal d2 entries -> k underflows to exactly 0
EPS_LN = 0.01  # ln(6.25*d2 + EPS_LN); guards d2 ~ -1e-4 roundoff on the diagonal

_CACHE = {}


def _build(G, P):
    import concourse.bacc as bacc
    import concourse.tile as tile
    from concourse import mybir

    f32 = mybir.dt.float32
    bf16 = mybir.dt.bfloat16
    AF = mybir.ActivationFunctionType
    OP = mybir.AluOpType

    T = G + P
    NT = T // 128  # t-chunks (partition dim of Kt)
    NG = G // 128  # gen-block t-chunks
    GS = G // 512  # 512-wide g slices
    assert T % 128 == 0 and G % 512 == 0 and P % 128 == 0

    nc = bacc.Bacc("TRN2", target_bir_lowering=False, debug=False, num_devices=8)

    ta = nc.dram_tensor("ta", [66, T], f32, kind="ExternalInput")
    gb = nc.dram_tensor("gb", [66, G], f32, kind="ExternalInput")
    posa = nc.dram_tensor("posa", [128, P // 128, 65], bf16, kind="ExternalInput")
    gena = nc.dram_tensor("gena", [128, G // 128, 65], bf16, kind="ExternalInput")
    ident = nc.dram_tensor("ident", [128, 128], bf16, kind="ExternalInput")
    bigi = nc.dram_tensor("bigi", [128, 128], bf16, kind="ExternalInput")
    ones_r = nc.dram_tensor("ones_r", [128, 1], bf16, kind="ExternalInput")
    ones_b = nc.dram_tensor("ones_b", [1, 128], f32, kind="ExternalInput")
    ones_v = nc.dram_tensor("ones_v", [64, 1], f32, kind="ExternalInput")
    outh = nc.dram_tensor("out", [1, 2], f32, kind="ExternalOutput")

    from contextlib import ExitStack

    with tile.TileContext(nc) as tc, ExitStack() as ctx:
        kpool = ctx.enter_context(tc.tile_pool(name="kpool", bufs=1))
        bigp = ctx.enter_context(tc.tile_pool(name="bigp", bufs=2))
        singles = ctx.enter_context(tc.tile_pool(name="singles", bufs=1))
        spool = ctx.enter_context(tc.tile_pool(name="spool", bufs=1))
        smalls = ctx.enter_context(tc.tile_pool(name="smalls", bufs=1))
        wpool = ctx.enter_context(tc.tile_pool(name="wpool", bufs=2))
        vtp = ctx.enter_context(tc.tile_pool(name="vtp", bufs=1))

        # persistent kernel matrix, [128, NT, G] bf16 (t-chunk major)
        KT = kpool.tile([128, NT, G], bf16)

        TAs = bigp.tile([66, T], f32, tag="b16", bufs=1)
        GBs = bigp.tile([66, G], f32, tag="b8", bufs=2)
        nc.sync.dma_start(out=TAs[:], in_=ta[:, :])
        nc.sync.dma_start(out=GBs[:], in_=gb[:, :])

        POSAs = singles.tile([128, P // 128, 65], bf16)
        GENAs = singles.tile([128, G // 128, 65], bf16)
        IDENTs = singles.tile([128, 128], bf16)
        BIGIs = singles.tile([128, 128], bf16)
        ONESRs = singles.tile([128, 1], bf16)
        ONESBs = singles.tile([1, 128], f32)
        ONESVs = singles.tile([64, 1], f32)
        nc.sync.dma_start(out=POSAs[:], in_=posa[:, :, :])
        nc.sync.dma_start(out=GENAs[:], in_=gena[:, :, :])
        nc.sync.dma_start(out=IDENTs[:], in_=ident[:, :])
        nc.sync.dma_start(out=BIGIs[:], in_=bigi[:, :])
        nc.sync.dma_start(out=ONESRs[:], in_=ones_r[:, :])
        nc.sync.dma_start(out=ONESBs[:], in_=ones_b[:, :])
        nc.sync.dma_start(out=ONESVs[:], in_=ones_v[:, :])

        colacc = smalls.tile([128, NT], f32)
        lnc = smalls.tile([128, NT], f32)
        bvec = smalls.tile([128, NT], f32)
        epsb = smalls.tile([128, 1], f32)
        qeps = smalls.tile([1, 1], f32)
        nc.vector.memset(epsb[:], EPS_LN)
        nc.vector.memset(qeps[:], 1.0e-35)

        # ---- Phase 1: build Kt, col sums (ACT accum), row sums (PE ones-matmul)
        with (
            tc.tile_pool(name="d2p", bufs=1, space="PSUM") as dp,
            tc.tile_pool(name="rap", bufs=1, space="PSUM") as rp,
        ):
            rowaccs = [
                rp.tile([1, 512], f32, tag=f"ra{j}", name=f"rowacc{j}")
                for j in range(GS)
            ]
            for i in range(NT):
                d2 = dp.tile([128, G], f32, tag="d2")
                jd = (i * 128) // 512 if i < NG else -1  # g-slice holding the diagonal
                for j in range(GS):
                    nc.tensor.matmul(
                        d2[:, j * 512 : (j + 1) * 512],
                        TAs[:, i * 128 : (i + 1) * 128],
                        GBs[:, j * 512 : (j + 1) * 512],
                        start=True,
                        stop=(j != jd),
                        skip_group_check=True,
                    )
                if jd >= 0:
                    nc.tensor.matmul(
                        d2[:, i * 128 : i * 128 + 128],
                        IDENTs[:],
                        BIGIs[:],
                        start=False,
                        stop=True,
                        skip_group_check=True,
                    )
                S = spool.tile([128, G], f32, tag="scratch")
                # u = ln(6.25*d2 + eps); v = exp(0.5*u) = 2.5*sqrt(d2+eps');
                # k = exp(-v) -> bf16, accumulating column sums per partition
                nc.scalar.activation(S[:], d2[:], AF.Ln, bias=epsb[:], scale=6.25)
                nc.scalar.activation(S[:], S[:], AF.Exp, scale=0.5)
                nc.scalar.activation(
                    KT[:, i, :], S[:], AF.Exp, scale=-1.0,
                    accum_out=colacc[:, i : i + 1],
                )
                for j in range(GS):
                    nc.tensor.matmul(
                        rowaccs[j][:],
                        ONESRs[:],
                        KT[:, i, j * 512 : (j + 1) * 512],
                        start=(i == 0),
                        stop=(i == NT - 1),
                        skip_group_check=True,
                    )
            # row sums -> a = rsqrt(row) (in place, single-partition vector)
            rowS = spool.tile([1, G], f32, tag="scratch")
            for j in range(GS):
                nc.scalar.copy(rowS[:, j * 512 : (j + 1) * 512], rowaccs[j][:])
            nc.scalar.activation(rowS[:], rowS[:], AF.Ln)
            nc.scalar.activation(rowS[:], rowS[:], AF.Exp, scale=-0.5)

        # b = rsqrt(col) per t (per-partition, [128, NT])
        nc.scalar.activation(lnc[:], colacc[:], AF.Ln)
        nc.scalar.activation(bvec[:], lnc[:], AF.Exp, scale=-0.5)

        # broadcast a across partitions -> [128, G] bf16
        BCA = singles.tile([128, G], bf16)
        with tc.tile_pool(name="bcp", bufs=2, space="PSUM") as bp:
            for j in range(GS):
                pb = bp.tile([128, 512], f32, tag="bc")
                nc.tensor.matmul(
                    pb[:], ONESBs[:], rowS[:, j * 512 : (j + 1) * 512],
                    start=True, stop=True, skip_group_check=True,
                )
                nc.vector.tensor_copy(BCA[:, j * 512 : (j + 1) * 512], pb[:])

        # ---- Phase 1.75 + 2a: nk = k*min(a*b, 1e6) (in place), side matmuls
        with tc.tile_pool(name="p2p", bufs=1, space="PSUM") as p2:
            psums = {}
            for side in range(2):
                for j in range(GS):
                    psums[(side, j)] = p2.tile(
                        [65, 512], f32, tag=f"p2_{side}_{j}", name=f"p2_{side}_{j}"
                    )
            for i in range(NT):
                W = wpool.tile([128, G], bf16, tag="w")
                nc.vector.tensor_scalar(
                    W[:], BCA[:], bvec[:, i : i + 1], 1.0e6, op0=OP.mult, op1=OP.min
                )
                nc.vector.tensor_mul(KT[:, i, :], KT[:, i, :], W[:])
                side = 0 if i < NG else 1
                lhs = GENAs[:, i, :] if side == 0 else POSAs[:, i - NG, :]
                first = i in (0, NG)
                last = i in (NG - 1, NT - 1)
                for j in range(GS):
                    nc.tensor.matmul(
                        psums[(side, j)][:],
                        lhs,
                        KT[:, i, j * 512 : (j + 1) * 512],
                        start=first,
                        stop=last,
                        skip_group_check=True,
                    )
            # rows 0:64 = M_neg.T / M_pos.T ; row 64 = s_gen / s_pos
            PNo = bigp.tile([65, G], f32, tag="b8", bufs=2)
            PPo = bigp.tile([65, G], f32, tag="b8", bufs=2)
            for j in range(GS):
                js = slice(j * 512, (j + 1) * 512)
                nc.scalar.copy(PNo[:, js], psums[(0, j)][:])
                nc.scalar.copy(PPo[:, js], psums[(1, j)][:])

        # ---- Phase 2b: V.T = bcast(s_gen)*M_pos.T - bcast(s_pos)*M_neg.T
        qS = spool.tile([1, G], f32, tag="scratch")
        with (
            tc.tile_pool(name="bc2", bufs=1, space="PSUM") as bp2,
            tc.tile_pool(name="qp", bufs=2, space="PSUM") as qp,
        ):
            for j in range(GS):
                js = slice(j * 512, (j + 1) * 512)
                sgr = vtp.tile([1, 512], f32, tag="sgr")
                spr = vtp.tile([1, 512], f32, tag="spr")
                nc.scalar.copy(sgr[:], PNo[64:65, js])
                nc.scalar.copy(spr[:], PPo[64:65, js])
                bg = bp2.tile([64, 512], f32, tag="bg")
                bpp = bp2.tile([64, 512], f32, tag="bp")
                nc.tensor.matmul(
                    bg[:], ONESBs[:, 0:64], sgr[:],
                    start=True, stop=True, skip_group_check=True,
                )
                nc.tensor.matmul(
                    bpp[:], ONESBs[:, 0:64], spr[:],
                    start=True, stop=True, skip_group_check=True,
                )
                vt1 = vtp.tile([64, 512], f32, tag="vt1")
                vt2 = vtp.tile([64, 512], f32, tag="vt2")
                nc.vector.tensor_mul(vt1[:], PPo[0:64, js], bg[:])
                nc.vector.tensor_mul(vt2[:], PNo[0:64, js], bpp[:])
                nc.vector.tensor_sub(vt1[:], vt1[:], vt2[:])
                nc.vector.tensor_mul(vt2[:], vt1[:], vt1[:])  # V^2
                qt = qp.tile([1, 512], f32, tag="q")
                nc.tensor.matmul(
                    qt[:], ONESVs[:], vt2[:], start=True, stop=True,
                    skip_group_check=True,
                )
                nc.scalar.copy(qS[:, js], qt[:])

        # loss partial = sum(q);  drift partial = sum(sqrt(q))
        lossP = smalls.tile([1, 1], f32)
        dnP = smalls.tile([1, 1], f32)
        outS = smalls.tile([1, 2], f32)
        nc.vector.tensor_reduce(lossP[:], qS[:], axis=mybir.AxisListType.X, op=OP.add)
        nc.scalar.activation(qS[:], qS[:], AF.Ln, bias=qeps[:])
        nc.scalar.activation(qS[:], qS[:], AF.Exp, scale=0.5)
        nc.vector.tensor_reduce(dnP[:], qS[:], axis=mybir.AxisListType.X, op=OP.add)
        nc.vector.tensor_copy(outS[:, 0:1], lossP[:])
        nc.vector.tensor_copy(outS[:, 1:2], dnP[:])
        nc.sync.dma_start(out=outh[:, :], in_=outS[:])

    nc.compile()
    return nc


def _prep_class(gen_c, pos_c):
    """Host-side input prep for one class -> dict of named arrays."""
    gen_c = np.ascontiguousarray(gen_c, dtype=np.float32)
    pos_c = np.ascontiguousarray(pos_c, dtype=np.float32)
    G, D = gen_c.shape
    P = pos_c.shape[0]
    T = G + P
    targets = np.concatenate([gen_c, pos_c], axis=0)

    ta = np.empty((66, T), np.float32)
    ta[0:64] = -2.0 * targets.T
    ta[64] = (targets * targets).sum(axis=1)
    ta[65] = 1.0

    gbm = np.empty((66, G), np.float32)
    gbm[0:64] = gen_c.T
    gbm[64] = 1.0
    gbm[65] = (gen_c * gen_c).sum(axis=1)

    def aug(x):
        n = x.shape[0]
        a = np.empty((n, 65), np.float32)
        a[:, 0:64] = x
        a[:, 64] = 1.0
        return (
            a.astype(ml_dtypes.bfloat16)
            .reshape(n // 128, 128, 65)
            .transpose(1, 0, 2)
            .copy()
        )

    bf = ml_dtypes.bfloat16
    return {
        "ta": ta,
        "gb": gbm,
        "posa": aug(pos_c),
        "gena": aug(gen_c),
        "ident": np.eye(128, dtype=bf),
        "bigi": (BIG * np.eye(128)).astype(bf),
        "ones_r": np.ones((128, 1), bf),
        "ones_b": np.ones((1, 128), np.float32),
        "ones_v": np.ones((64, 1), np.float32),
    }


def kernel(generated, labels_gen, positive, labels_pos):
    from concourse.bass_utils import run_bass_kernel_spmd

    generated = np.asarray(generated, dtype=np.float32)
    positive = np.asarray(positive, dtype=np.float32)
    N, D = generated.shape
    Np = positive.shape[0]
    G, P = N // C, Np // C
    assert D == 64

    key = (G, P)
    if key not in _CACHE:
        _CACHE[key] = _build(G, P)
    nc = _CACHE[key]

    in_maps = [
        _prep_class(
            generated[c * G : (c + 1) * G], positive[c * P : (c + 1) * P]
        )
        for c in range(C)
    ]
    res = run_bass_kernel_spmd(nc, in_maps, core_ids=list(range(C)))
    sums = np.stack([res.results[i]["out"][0] for i in range(C)])  # [C, 2]
    loss = sums[:, 0].sum() / (N * D)
    dn = sums[:, 1].sum() / N
    return np.float32(loss), np.float32(dn)


if __name__ == "__main__":
    rng = np.random.default_rng(0)
    N = 16384
    gen = rng.standard_normal((N, 64), dtype=np.float32)
    pos = rng.standard_normal((N, 64), dtype=np.float32)
    lg = np.repeat(np.arange(C), N // C).astype(np.int32)
    print(kernel(gen, lg, pos, lg))


# revision 34
# speedup vs baseline: 1.2813x; 1.2813x over previous
"""Trainium2 Bass kernel for nn_ClassConditionalDriftingLoss.

Math per class c (G gen rows, P pos rows, D=64, T=G+P targets):
  d2[t,g]  = ||x_t||^2 + ||y_g||^2 - 2 x_t.y_g          (x=targets, y=gen)
  k        = exp(-2.5*sqrt(d2)),  k[diag]=0             (dist normalized by sqrt(D)=8,
                                                         TEMP=0.05 -> exp(-20*dist/8*... ) = exp(-2.5*sqrt(d2)))
  row[g]   = sum_t k[g,t];  col[t] = sum_g k[g,t]
  nk       = k * min(rsqrt(row[g])*rsqrt(col[t]), 1e6)  (== k / sqrt(max(row*col,1e-12)))
  s_gen[g] = sum_{t<G} nk ; s_pos[g] = sum_{t>=G} nk
  M_pos    = nk[:,G:] @ pos ; M_neg = nk[:,:G] @ gen
  V        = s_gen*M_pos - s_pos*M_neg
  loss    += sum(V^2);  drift += sum_g ||V[g]||

The kernel matrix is held transposed (Kt[t,g], t on partitions) in SBUF bf16.
One class per NeuronCore (8 classes / 8 cores), no collectives; host sums the
8 scalar pairs.

exp/ln only (single ACT table set `natural_log_exp_and_others`):
  sqrt(z) = exp(0.5*ln(z)); rsqrt(z) = exp(-0.5*ln(z)).
Diagonal masking: a BIG*I matmul accumulated onto the d2 PSUM drives those
entries to exp(-sqrt(6.25e9)) == 0.
"""

import sys

for _p in ("/opt/trn_rl_repo", "/root/.axon_site/_ro/trn_rl_repo"):
    if _p not in sys.path:
        sys.path.insert(0, _p)

import math

import ml_dtypes
import numpy as np

C = 8
BIG = 1.0e9  # added to diagonal d2 entries -> k underflows to exactly 0
EPS_LN = 0.01  # ln(6.25*d2 + EPS_LN); guards d2 ~ -1e-4 roundoff on the diagonal

_CACHE = {}


def _patch_act_tables():
    """Keep every ACT func only in natural_log_exp_and_others so the
    table-load inserter never thrashes between sets (Ln/Exp/Copy all live
    there; set IDs stay aligned with the compiler's act_info.json)."""
    import functools

    import concourse.bacc as bacc
    import concourse.hw_specs as hw_specs

    if getattr(hw_specs.get_activation_tables, "_drift_patched", False):
        return
    orig = hw_specs.get_activation_tables

    @functools.cache
    def patched(module_arch):
        keep = "natural_log_exp_and_others"
        return {
            name: (funcs if name == keep else set())
            for name, funcs in orig(module_arch).items()
        }

    patched._drift_patched = True
    hw_specs.get_activation_tables = patched
    bacc.get_activation_tables = patched


def _build(G, P):
    import concourse.bacc as bacc
    import concourse.tile as tile
    from concourse import mybir

    _patch_act_tables()

    f32 = mybir.dt.float32
    bf16 = mybir.dt.bfloat16
    AF = mybir.ActivationFunctionType
    OP = mybir.AluOpType

    T = G + P
    NT = T // 128  # t-chunks (partition dim of Kt)
    NG = G // 128  # gen-block t-chunks
    GS = G // 512  # 512-wide g slices
    RW = 512  # matmul slice width (psum bank limit: 512 fp32)
    GR = G // RW
    assert T % 128 == 0 and G % 512 == 0 and P % 128 == 0

    nc = bacc.Bacc("TRN2", target_bir_lowering=False, debug=False, num_devices=8)

    ta = nc.dram_tensor("ta", [66, T], f32, kind="ExternalInput")
    gb = nc.dram_tensor("gb", [66, G], f32, kind="ExternalInput")
    posa = nc.dram_tensor("posa", [128, P // 128, 65], bf16, kind="ExternalInput")
    gena = nc.dram_tensor("gena", [128, G // 128, 65], bf16, kind="ExternalInput")
    ident = nc.dram_tensor("ident", [128, 128], bf16, kind="ExternalInput")
    identf = nc.dram_tensor("identf", [128, 128], f32, kind="ExternalInput")
    bigi = nc.dram_tensor("bigi", [128, 128], bf16, kind="ExternalInput")
    ones_r = nc.dram_tensor("ones_r", [128, 1], bf16, kind="ExternalInput")
    ones_b = nc.dram_tensor("ones_b", [1, 128], f32, kind="ExternalInput")
    ones_v = nc.dram_tensor("ones_v", [64, 1], f32, kind="ExternalInput")
    outh = nc.dram_tensor("out", [1, 2], f32, kind="ExternalOutput")

    from contextlib import ExitStack

    with tile.TileContext(nc) as tc, ExitStack() as ctx:
        kpool = ctx.enter_context(tc.tile_pool(name="kpool", bufs=1))
        bigp = ctx.enter_context(tc.tile_pool(name="bigp", bufs=2))
        singles = ctx.enter_context(tc.tile_pool(name="singles", bufs=1))
        spool = ctx.enter_context(tc.tile_pool(name="spool", bufs=1))
        smalls = ctx.enter_context(tc.tile_pool(name="smalls", bufs=1))
        wpool = ctx.enter_context(tc.tile_pool(name="wpool", bufs=2))
        vtp = ctx.enter_context(tc.tile_pool(name="vtp", bufs=1))

        # persistent kernel matrix, [128, NT, G] bf16 (t-chunk major)
        KT = kpool.tile([128, NT, G], bf16)

        TAs = bigp.tile([66, T], f32, tag="b16", bufs=1)
        GBs = bigp.tile([66, G], f32, tag="b8", bufs=2)
        # spread input DMAs across engine queues so they overlap
        nc.sync.dma_start(out=TAs[:, : T // 2], in_=ta[:, : T // 2])
        nc.scalar.dma_start(out=TAs[:, T // 2 :], in_=ta[:, T // 2 :])
        nc.gpsimd.dma_start(out=GBs[:], in_=gb[:, :])

        POSAs = singles.tile([128, P // 128, 65], bf16)
        GENAs = singles.tile([128, G // 128, 65], bf16)
        IDENTs = singles.tile([128, 128], bf16)
        IDENTFs = singles.tile([128, 128], f32)
        BIGIs = singles.tile([128, 128], bf16)
        ONESRs = singles.tile([128, 1], bf16)
        ONESBs = singles.tile([1, 128], f32)
        ONESVs = singles.tile([64, 1], f32)
        nc.gpsimd.dma_start(out=POSAs[:], in_=posa[:, :, :])
        nc.gpsimd.dma_start(out=GENAs[:], in_=gena[:, :, :])
        nc.scalar.dma_start(out=IDENTs[:], in_=ident[:, :])
        nc.scalar.dma_start(out=IDENTFs[:], in_=identf[:, :])
        nc.scalar.dma_start(out=BIGIs[:], in_=bigi[:, :])
        nc.sync.dma_start(out=ONESRs[:], in_=ones_r[:, :])
        nc.sync.dma_start(out=ONESBs[:], in_=ones_b[:, :])
        nc.sync.dma_start(out=ONESVs[:], in_=ones_v[:, :])

        colacc = smalls.tile([128, NT], f32)
        lnc = smalls.tile([128, NT], f32)
        bvec = smalls.tile([128, NT], f32)
        epsb = smalls.tile([128, 1], f32)
        qeps = smalls.tile([1, 1], f32)
        nc.vector.memset(epsb[:], EPS_LN)
        nc.vector.memset(qeps[:], 1.0e-35)

        # ---- Phase 1: build Kt, col sums (ACT accum), row sums (PE ones-matmul)
        with (
            tc.tile_pool(name="d2p", bufs=1, space="PSUM") as dp,
            tc.tile_pool(name="rap", bufs=1, space="PSUM") as rp,
        ):
            rowaccs = [
                rp.tile([1, RW], f32, tag=f"ra{j}", name=f"rowacc{j}")
                for j in range(GR)
            ]
            # pos chunks first so each rowacc bank opens with a full-width
            # start=True matmul; gen-chunk transposes then accumulate slices
            for i in list(range(NG, NT)) + list(range(NG)):
                d2 = dp.tile([128, G], f32, tag="d2")
                jd = (i * 128) // 512 if i < NG else -1  # g-slice holding the diagonal
                for j in range(GS):
                    nc.tensor.matmul(
                        d2[:, j * 512 : (j + 1) * 512],
                        TAs[:, i * 128 : (i + 1) * 128],
                        GBs[:, j * 512 : (j + 1) * 512],
                        start=True,
                        stop=(j != jd),
                        skip_group_check=True,
                    )
                if jd >= 0:
                    nc.tensor.matmul(
                        d2[:, i * 128 : i * 128 + 128],
                        IDENTs[:],
                        BIGIs[:],
                        start=False,
                        stop=True,
                        skip_group_check=True,
                    )
                S = spool.tile([128, G], f32, tag="scratch")
                # u = ln(6.25*d2 + eps); v = exp(0.5*u) = 2.5*sqrt(d2+eps');
                # k = exp(-v) -> bf16, accumulating column sums per partition
                nc.scalar.activation(S[:], d2[:], AF.Ln, bias=epsb[:], scale=6.25)
                nc.scalar.activation(S[:], S[:], AF.Exp, scale=0.5)
                nc.scalar.activation(
                    KT[:, i, :], S[:], AF.Exp, scale=-1.0,
                    accum_out=colacc[:, i : i + 1],
                )
                # row sums: pos block via ones-matmul; gen block comes free
                # from colacc (gen-gen block of Kt is symmetric:
                # row_gen[128i+p] == colacc[p, i]), transposed into the same
                # accumulators by a colacc[:, i] x identity matmul.
                if i < NG:
                    j, m = divmod(i * 128, RW)
                    nc.tensor.matmul(
                        rowaccs[j][:, m : m + 128],
                        colacc[:, i : i + 1],
                        IDENTFs[:],
                        start=False,
                        stop=(m + 128 == RW),
                        skip_group_check=True,
                    )
                else:
                    for j in range(GR):
                        nc.tensor.matmul(
                            rowaccs[j][:],
                            ONESRs[:],
                            KT[:, i, j * RW : (j + 1) * RW],
                            start=(i == NG),
                            stop=False,
                            skip_group_check=True,
                        )
            # row sums -> a = rsqrt(row) (in place, single-partition vector)
            rowS = spool.tile([1, G], f32, tag="scratch")
            for j in range(GR):
                nc.vector.tensor_copy(
                    rowS[:, j * RW : (j + 1) * RW], rowaccs[j][:]
                )
            nc.scalar.activation(rowS[:], rowS[:], AF.Ln)
            nc.scalar.activation(rowS[:], rowS[:], AF.Exp, scale=-0.5)

        # b = rsqrt(col) per t (per-partition, [128, NT])
        nc.scalar.activation(lnc[:], colacc[:], AF.Ln)
        nc.scalar.activation(bvec[:], lnc[:], AF.Exp, scale=-0.5)

        # broadcast a across partitions -> [128, G] bf16
        BCA = singles.tile([128, G], bf16)
        with tc.tile_pool(name="bcp", bufs=2, space="PSUM") as bp:
            for j in range(GS):
                pb = bp.tile([128, 512], f32, tag="bc")
                nc.tensor.matmul(
                    pb[:], ONESBs[:], rowS[:, j * 512 : (j + 1) * 512],
                    start=True, stop=True, skip_group_check=True,
                )
                nc.vector.tensor_copy(BCA[:, j * 512 : (j + 1) * 512], pb[:])

        # ---- Phase 1.75 + 2a: nk = k*min(a*b, 1e6) (in place), side matmuls
        with tc.tile_pool(name="p2p", bufs=1, space="PSUM") as p2:
            psums = {}
            for side in range(2):
                for j in range(GR):
                    psums[(side, j)] = p2.tile(
                        [65, RW], f32, tag=f"p2_{side}_{j}", name=f"p2_{side}_{j}"
                    )
            for i in range(NT):
                W = wpool.tile([128, G], bf16, tag="w")
                nc.vector.tensor_scalar(
                    W[:], BCA[:], bvec[:, i : i + 1], 1.0e6, op0=OP.mult, op1=OP.min
                )
                nc.vector.tensor_mul(KT[:, i, :], KT[:, i, :], W[:])
                side = 0 if i < NG else 1
                lhs = GENAs[:, i, :] if side == 0 else POSAs[:, i - NG, :]
                first = i in (0, NG)
                last = i in (NG - 1, NT - 1)
                for j in range(GR):
                    nc.tensor.matmul(
                        psums[(side, j)][:],
                        lhs,
                        KT[:, i, j * RW : (j + 1) * RW],
                        start=first,
                        stop=last,
                        skip_group_check=True,
                    )
            # rows 0:64 = M_neg.T / M_pos.T ; row 64 = s_gen / s_pos
            PNo = bigp.tile([65, G], f32, tag="b8", bufs=2)
            PPo = bigp.tile([65, G], f32, tag="b8", bufs=2)
            for j in range(GR):
                js = slice(j * RW, (j + 1) * RW)
                nc.vector.tensor_copy(PNo[:, js], psums[(0, j)][:])
                nc.vector.tensor_copy(PPo[:, js], psums[(1, j)][:])

        # ---- Phase 2b: V.T = bcast(s_gen)*M_pos.T - bcast(s_pos)*M_neg.T
        qS = spool.tile([1, G], f32, tag="scratch")
        with (
            tc.tile_pool(name="bc2", bufs=1, space="PSUM") as bp2,
            tc.tile_pool(name="qp", bufs=2, space="PSUM") as qp,
        ):
            for j in range(GS):
                js = slice(j * 512, (j + 1) * 512)
                sgr = vtp.tile([1, 512], f32, tag="sgr")
                spr = vtp.tile([1, 512], f32, tag="spr")
                nc.vector.tensor_copy(sgr[:], PNo[64:65, js])
                nc.vector.tensor_copy(spr[:], PPo[64:65, js])
                bg = bp2.tile([64, 512], f32, tag="bg")
                bpp = bp2.tile([64, 512], f32, tag="bp")
                nc.tensor.matmul(
                    bg[:], ONESBs[:, 0:64], sgr[:],
                    start=True, stop=True, skip_group_check=True,
                )
                nc.tensor.matmul(
                    bpp[:], ONESBs[:, 0:64], spr[:],
                    start=True, stop=True, skip_group_check=True,
                )
                vt1 = vtp.tile([64, 512], f32, tag="vt1")
                vt2 = vtp.tile([64, 512], f32, tag="vt2")
                nc.vector.tensor_mul(vt1[:], PPo[0:64, js], bg[:])
                nc.vector.tensor_mul(vt2[:], PNo[0:64, js], bpp[:])
                nc.vector.tensor_sub(vt1[:], vt1[:], vt2[:])
                nc.vector.tensor_mul(vt2[:], vt1[:], vt1[:])  # V^2
                qt = qp.tile([1, 512], f32, tag="q")
                nc.tensor.matmul(
                    qt[:], ONESVs[:], vt2[:], start=True, stop=True,
                    skip_group_check=True,
                )
                nc.vector.tensor_copy(qS[:, js], qt[:])

        # loss partial = sum(q);  drift partial = sum(sqrt(q))
        lossP = smalls.tile([1, 1], f32)
        dnP = smalls.tile([1, 1], f32)
        outS = smalls.tile([1, 2], f32)
        nc.vector.tensor_reduce(lossP[:], qS[:], axis=mybir.AxisListType.X, op=OP.add)
        nc.scalar.activation(qS[:], qS[:], AF.Ln, bias=qeps[:])
        nc.scalar.activation(qS[:], qS[:], AF.Exp, scale=0.5)
        nc.vector.tensor_reduce(dnP[:], qS[:], axis=mybir.AxisListType.X, op=OP.add)
        nc.vector.tensor_copy(outS[:, 0:1], lossP[:])
        nc.vector.tensor_copy(outS[:, 1:2], dnP[:])
        nc.sync.dma_start(out=outh[:, :], in_=outS[:])

    nc.compile()
    return nc


def _prep_class(gen_c, pos_c):
    """Host-side input prep for one class -> dict of named arrays."""
    gen_c = np.ascontiguousarray(gen_c, dtype=np.float32)
    pos_c = np.ascontiguousarray(pos_c, dtype=np.float32)
    G, D = gen_c.shape
    P = pos_c.shape[0]
    T = G + P
    targets = np.concatenate([gen_c, pos_c], axis=0)

    ta = np.empty((66, T), np.float32)
    ta[0:64] = -2.0 * targets.T
    ta[64] = (targets * targets).sum(axis=1)
    ta[65] = 1.0

    gbm = np.empty((66, G), np.float32)
    gbm[0:64] = gen_c.T
    gbm[64] = 1.0
    gbm[65] = (gen_c * gen_c).sum(axis=1)

    def aug(x):
        n = x.shape[0]
        a = np.empty((n, 65), np.float32)
        a[:, 0:64] = x
        a[:, 64] = 1.0
        return (
            a.astype(ml_dtypes.bfloat16)
            .reshape(n // 128, 128, 65)
            .transpose(1, 0, 2)
            .copy()
        )

    bf = ml_dtypes.bfloat16
    return {
        "ta": ta,
        "gb": gbm,
        "posa": aug(pos_c),
        "gena": aug(gen_c),
        "ident": np.eye(128, dtype=bf),
        "identf": np.eye(128, dtype=np.float32),
        "bigi": (BIG * np.eye(128)).astype(bf),
        "ones_r": np.ones((128, 1), bf),
        "ones_b": np.ones((1, 128), np.float32),
        "ones_v": np.ones((64, 1), np.float32),
    }


def kernel(generated, labels_gen, positive, labels_pos):
    from concourse.bass_utils import run_bass_kernel_spmd

    generated = np.asarray(generated, dtype=np.float32)
    positive = np.asarray(positive, dtype=np.float32)
    N, D = generated.shape
    Np = positive.shape[0]
    G, P = N // C, Np // C
    assert D == 64

    key = (G, P)
    if key not in _CACHE:
        _CACHE[key] = _build(G, P)
    nc = _CACHE[key]

    in_maps = [
        _prep_class(
            generated[c * G : (c + 1) * G], positive[c * P : (c + 1) * P]
        )
        for c in range(C)
    ]
    res = run_bass_kernel_spmd(nc, in_maps, core_ids=list(range(C)))
    sums = np.stack([res.results[i]["out"][0] for i in range(C)])  # [C, 2]
    loss = sums[:, 0].sum() / (N * D)
    dn = sums[:, 1].sum() / N
    return np.float32(loss), np.float32(dn)


if __name__ == "__main__":
    rng = np.random.default_rng(0)
    N = 16384
    gen = rng.standard_normal((N, 64), dtype=np.float32)
    pos = rng.standard_normal((N, 64), dtype=np.float32)
    lg = np.repeat(np.arange(C), N // C).astype(np.int32)
    print(kernel(gen, lg, pos, lg))
